# revision 1
# baseline (speedup 1.0000x reference)
"""Trainium2 Bass kernel for a directed MPNN layer (8 NeuronCores, SPMD).

Reference computation (per edge e = (src, tgt)):
    msg  = relu(edge_hidden @ W_msg.T + b_msg)                     (E, H)
    agg  = segment_sum(msg, tgt, N)                                (N, H)
    excl[e] = sum msg[f] over f with (tgt_f, src_f) == (src_e, tgt_e)
    out[e]  = relu(x[src_e] @ Wx.T + edge_attr[e] @ Wa.T
                   + (agg[src_e] - excl[e]) @ Wm.T + b_upd)
  with W_upd = [Wx | Wa | Wm] split along columns (64 | 16 | 64).

Decomposition (no cross-core communication at all):
    node_term[v] = x[v] @ Wx.T + agg[v] @ Wm.T + b_upd
    out[e] = relu(node_term[src_e] + edge_attr[e] @ Wa.T - excl[e] @ Wm.T)

  Each core owns 5000 nodes. Edges are reverse pairs (e <-> e +/- E/2),
  so for out-edge e = rev(f), excl[e] = msg[f] (plus rare duplicate-pair
  corrections) and src_e = tgt_f. Sorting in-edges by tgt gives one
  stream that serves both passes:
    pass 1: msg(f) -> one-hot matmul segment-sum -> agg -> node_term
    pass 2: out[rev(f)] = relu(nt[tgt_f] + attrW[rev(f)] - msg(f)@Wm.T)
  node_term rows are delivered by a host-built one-hot matmul (U2), so
  there are no gathers. ~500 duplicate-pair corrections go through 128
  "special" node_term rows computed on device and a fix-up group whose
  outputs the host splices in.

Matmul dtypes: bf16 for the big per-edge streams (inputs host-cast),
float32r (1.6e-4) for node_term math. All accumulation is fp32 PSUM.
"""

import numpy as np
import ml_dtypes

import concourse.bacc as bacc
import concourse.bass as bass
import concourse.mybir as mybir
import concourse.tile as tile
from concourse.bass_utils import run_bass_kernel_spmd

F32 = mybir.dt.float32
F32R = mybir.dt.float32r
BF16 = mybir.dt.bfloat16
I32 = mybir.dt.int32
ALU = mybir.AluOpType
ACTF = mybir.ActivationFunctionType
NPBF = ml_dtypes.bfloat16

N = 40000
E = 800000
E2 = E // 2
H = 64
A = 16
NC = 8
P = 128

NPC = N // NC           # 5000 nodes per core
NBLK = 40               # 128-node blocks per core
NPC_PAD = NBLK * P      # 5120
SPEC_CAP = P            # special (correction) rows per core
NT_ROWS = NPC_PAD + SPEC_CAP

_CACHE = {}
_DEBUG_NT = False


def _build(k_blk: int):
    nch = NBLK * k_blk              # chunks per core (both passes)
    l1 = nch * P                    # padded edges per core
    assert nch % 2 == 0
    hch = nch // 2                  # chunks per partition-half of eh

    nc = bacc.Bacc("TRN2", target_bir_lowering=False, debug=False,
                   num_devices=NC)

    def inp(name, shape, dtype):
        return nc.dram_tensor(name, shape, dtype, kind="ExternalInput").ap()

    # eh (in-edges, tgt-sorted, feature-major): chunks 0..hch-1 on
    # partitions 0:64, chunks hch.. on partitions 64:128.
    eh2 = inp("eh2", [P, hch * P], BF16)
    tgt_rel = inp("tgt_rel", [P, nch], F32)
    attr_T = inp("attr_T", [A, l1], BF16)      # edge_attr of rev(f), T
    U2 = inp("U2", [P, l1], BF16)              # one-hot src_rel columns
    xT_own = inp("xT_own", [H, NPC_PAD], F32R)
    ehF_T = inp("ehF_T", [H, P], BF16)         # correction source rows
    ehRF_T = inp("ehRF_T", [H, P], BF16)       # eh[rev(affected e)], T
    attrF_T = inp("attrF_T", [A, P], BF16)
    Sneg = inp("Sneg", [P, P], F32R)
    didx = inp("didx", [P, 1], I32)
    Wmsg2 = inp("Wmsg2", [P, H], BF16)         # W_msg.T doubled (2x64)
    Wua = inp("Wua", [A, H], BF16)
    negWum = inp("negWum", [H, H], BF16)
    Wstack = inp("Wstack", [H + A, H], BF16)   # [negWum ; Wua]
    Wum = inp("Wum", [H, H], F32R)
    Wux = inp("Wux", [H, H], F32R)
    bupd = inp("bupd", [1, H], F32R)
    ones1 = inp("ones1", [1, P], F32R)
    ident = inp("ident", [P, P], BF16)
    iota4 = inp("iota4", [P, 4 * P], BF16)

    outT = nc.dram_tensor("outT", [H, l1 + P], F32, kind="ExternalOutput").ap()
    nt_own = nc.dram_tensor("nt_own", [NT_ROWS, P], BF16).ap()
    nt_dump = (nc.dram_tensor("nt_dump", [NT_ROWS, P], BF16,
                              kind="ExternalOutput").ap()
               if _DEBUG_NT else None)

    with tile.TileContext(nc) as tc:
        with (
            tc.tile_pool(name="const", bufs=1) as cst,
            tc.tile_pool(name="sb", bufs=3) as sb,
            tc.tile_pool(name="stage", bufs=3) as stg,
            tc.tile_pool(name="ps_msg", bufs=2, space="PSUM") as ps_msg,
            tc.tile_pool(name="ps_agg", bufs=2, space="PSUM") as ps_agg,
            tc.tile_pool(name="ps_m", bufs=2, space="PSUM") as ps_m,
            tc.tile_pool(name="ps_o", bufs=2, space="PSUM") as ps_o,
        ):
            def load_const(name, ap_in, shape, dtype):
                t = cst.tile(shape, dtype, tag=name)
                nc.sync.dma_start(t[:], ap_in[:])
                return t

            eh_sb = load_const("c_eh2", eh2, [P, hch * P], BF16)
            tgt_rel_sb = load_const("c_tgtrel", tgt_rel, [P, nch], F32)
            xT_sb = load_const("c_xt", xT_own, [H, NPC_PAD], F32R)
            Wmsg2_sb = load_const("c_wmsg2", Wmsg2, [P, H], BF16)
            Wua_sb = load_const("c_wua", Wua, [A, H], BF16)
            negWum_sb = load_const("c_nwum", negWum, [H, H], BF16)
            Wstack_sb = load_const("c_wstack", Wstack, [H + A, H], BF16)
            Wum_sb = load_const("c_wum", Wum, [H, H], F32R)
            Wux_sb = load_const("c_wux", Wux, [H, H], F32R)
            bupd_sb = load_const("c_bupd", bupd, [1, H], F32R)
            ones1_sb = load_const("c_ones1", ones1, [1, P], F32R)
            ident_sb = load_const("c_ident", ident, [P, P], BF16)
            iota4_sb = load_const("c_iota4", iota4, [P, 4 * P], BF16)
            Sneg_sb = load_const("c_sneg", Sneg, [P, P], F32R)
            didx_sb = load_const("c_didx", didx, [P, 1], I32)
            ehF_sb = load_const("c_ehf", ehF_T, [H, P], BF16)
            ehRF_sb = load_const("c_ehrf", ehRF_T, [H, P], BF16)
            attrF_sb = load_const("c_attrf", attrF_T, [A, P], BF16)

            def ehsl(ch, w=P):
                half, col = (0, ch) if ch < hch else (64, ch - hch)
                return eh_sb[half:half + H, col * P:col * P + w]

            def wmsl(ch):
                half = 0 if ch < hch else 64
                return Wmsg2_sb[half:half + H, :]

            # b_upd broadcast to 128 partitions via K=1 matmul
            ps_b = ps_agg.tile([P, H], F32, tag="agg")
            nc.tensor.matmul(ps_b[:], lhsT=ones1_sb[:],
                             rhs=bupd_sb[:],
                             start=True, stop=True)
            b_bcast = cst.tile([P, H], F32, tag="c_bb")
            nc.vector.tensor_copy(b_bcast[:], ps_b[:])

            # ---- pass 1: msg -> agg -> node_term, per 128-node block ----
            for b in range(NBLK):
                agg_ps = ps_agg.tile([H, P], F32, tag="agg")
                i = 0
                while i < k_blk:
                    gw = min(4, k_blk - i)
                    msg4_ps = ps_msg.tile([P, 4 * H], F32, tag="msg")
                    for j in range(gw):
                        ch = b * k_blk + i + j
                        nc.tensor.matmul(msg4_ps[:, j * H:(j + 1) * H],
                                         lhsT=ehsl(ch), rhs=wmsl(ch),
                                         start=True, stop=True)
                    msg4_sb = sb.tile([P, 4 * H], BF16, tag="msg_sb")
                    nc.vector.tensor_scalar(out=msg4_sb[:, :gw * H],
                                            in0=msg4_ps[:, :gw * H],
                                            scalar1=0.0, scalar2=None,
                                            op0=ALU.max)
                    ch0 = b * k_blk + i
                    t4_sb = sb.tile([P, 4 * P], BF16, tag="t_sb")
                    trs = tgt_rel_sb[:, ch0:ch0 + gw]
                    tr_bc = bass.AP(trs.tensor, trs.offset,
                                    trs.ap[:1] + [[1, gw], [0, P]])
                    nc.vector.tensor_tensor(out=t4_sb[:, :gw * P],
                                            in0=iota4_sb[:, :gw * P],
                                            in1=tr_bc, op=ALU.is_equal)
                    for j in range(gw):
                        nc.tensor.matmul(agg_ps[:],
                                         lhsT=msg4_sb[:, j * H:(j + 1) * H],
                                         rhs=t4_sb[:, j * P:(j + 1) * P],
                                         start=(i + j == 0),
                                         stop=(i + j == k_blk - 1))
                    i += gw
                aggT_sb = sb.tile([H, P], F32R, tag="aggT_sb")
                nc.vector.tensor_copy(aggT_sb[:], agg_ps[:])
                nt_ps = ps_msg.tile([P, H], F32, tag="msg")
                nc.tensor.matmul(nt_ps[:], lhsT=aggT_sb[:],
                                 rhs=Wum_sb[:],
                                 start=True, stop=False)
                nc.tensor.matmul(nt_ps[:],
                                 lhsT=xT_sb[:, b * P:(b + 1) * P],
                                 rhs=Wux_sb[:],
                                 start=False, stop=True)
                nt_sb = sb.tile([P, P], BF16, tag="nt_sb")
                nc.gpsimd.memset(nt_sb[:, H:], 0.0)
                nc.vector.tensor_tensor(out=nt_sb[:, 0:H], in0=nt_ps[:],
                                        in1=b_bcast[:], op=ALU.add)
                nc.sync.dma_start(nt_own[b * P:(b + 1) * P, :], nt_sb[:])

            # ---- special (correction) rows ----
            mF_ps = ps_m.tile([H, P], F32, tag="m")
            nc.tensor.matmul(mF_ps[:], lhsT=Wmsg2_sb[0:H, :], rhs=ehF_sb[:],
                             start=True, stop=True)
            mFT_sb = sb.tile([H, P], F32R, tag="mFT_sb")
            nc.vector.tensor_scalar(out=mFT_sb[:], in0=mF_ps[:], scalar1=0.0,
                                    scalar2=None, op0=ALU.max)
            mV_ps = ps_msg.tile([P, H], F32, tag="msg")
            nc.tensor.matmul(mV_ps[:], lhsT=mFT_sb[:],
                             rhs=Wum_sb[:],
                             start=True, stop=True)
            mV_sb = sb.tile([P, H], F32R, tag="mV_sb")
            nc.vector.tensor_copy(mV_sb[:], mV_ps[:])
            ntgD_sb = sb.tile([P, P], BF16, tag="ntgD_sb")
            nc.gpsimd.indirect_dma_start(
                out=ntgD_sb[:], out_offset=None, in_=nt_own[:],
                in_offset=bass.IndirectOffsetOnAxis(ap=didx_sb[:, 0:1], axis=0),
            )
            ntgD_f = sb.tile([P, H], F32, tag="ntgD_f")
            nc.vector.tensor_copy(ntgD_f[:], ntgD_sb[:, 0:H])
            spec_ps = ps_agg.tile([P, H], F32, tag="agg")
            nc.tensor.matmul(spec_ps[:], lhsT=Sneg_sb[:],
                             rhs=mV_sb[:],
                             start=True, stop=True)
            spec_sb = sb.tile([P, P], BF16, tag="spec_sb")
            nc.gpsimd.memset(spec_sb[:, H:], 0.0)
            nc.vector.tensor_tensor(out=spec_sb[:, 0:H], in0=spec_ps[:],
                                    in1=ntgD_f[:], op=ALU.add)
            nc.sync.dma_start(nt_own[NPC_PAD:NPC_PAD + SPEC_CAP, :],
                              spec_sb[:])

            if nt_dump is not None:
                nc.sync.dma_start(nt_dump[:], nt_own[:])

            # ---- pass 2: out[rev(f)] per block, groups of <=4 chunks ----
            # stacked rhs: partitions 0:64 = relu(msg_rev)T, 64:80 = attrT
            for b in range(NBLK):
                ntb_sb = sb.tile([P, P], BF16, tag="ntb")
                nc.sync.dma_start(ntb_sb[:], nt_own[b * P:(b + 1) * P, :])
                i = 0
                while i < k_blk:
                    gw = min(4, k_blk - i)          # chunks in this group
                    w = gw * P
                    ch0 = b * k_blk + i
                    c0 = ch0 * P
                    m_ps = ps_m.tile([H, 4 * P], F32, tag="m")
                    nc.tensor.matmul(m_ps[:, 0:w], lhsT=wmsl(ch0),
                                     rhs=ehsl(ch0, w), start=True, stop=True)
                    sx_sb = stg.tile([H + A, 4 * P], BF16, tag="sx")
                    nc.scalar.activation(sx_sb[0:H, 0:w], m_ps[:, 0:w],
                                         ACTF.Relu)
                    nc.scalar.dma_start(sx_sb[H:H + A, 0:w],
                                        attr_T[:, c0:c0 + w])
                    u2_sb = stg.tile([P, 4 * P], BF16, tag="u2")
                    nc.sync.dma_start(u2_sb[:, 0:w], U2[:, c0:c0 + w])
                    o_ps = ps_o.tile([H, 4 * P], F32, tag="o")
                    nc.tensor.matmul(o_ps[:, 0:w], lhsT=Wstack_sb[:],
                                     rhs=sx_sb[:, 0:w],
                                     start=True, stop=False)
                    nc.tensor.matmul(o_ps[:, 0:w], lhsT=ntb_sb[:, 0:H],
                                     rhs=u2_sb[:, 0:w],
                                     start=False, stop=True)
                    outT_sb = sb.tile([H, 4 * P], F32, tag="outT")
                    nc.vector.tensor_scalar(out=outT_sb[:, 0:w],
                                            in0=o_ps[:, 0:w], scalar1=0.0,
                                            scalar2=None, op0=ALU.max)
                    nc.scalar.dma_start(outT[:, c0:c0 + w], outT_sb[:, 0:w])
                    i += gw

            # ---- fix-up group for the corrected edges ----
            ntf_sb = sb.tile([P, P], BF16, tag="ntb")
            nc.sync.dma_start(ntf_sb[:], nt_own[NPC_PAD:NPC_PAD + P, :])
            mf_ps = ps_m.tile([H, 4 * P], F32, tag="m")
            nc.tensor.matmul(mf_ps[:, 0:P], lhsT=Wmsg2_sb[0:H, :],
                             rhs=ehRF_sb[:], start=True, stop=True)
            mfT_sb = sb.tile([H, 4 * P], BF16, tag="mrevT")
            nc.scalar.activation(mfT_sb[:, 0:P], mf_ps[:, 0:P], ACTF.Relu)
            of_ps = ps_o.tile([H, 4 * P], F32, tag="o")
            nc.tensor.matmul(of_ps[:, 0:P], lhsT=Wua_sb[:], rhs=attrF_sb[:],
                             start=True, stop=False)
            nc.tensor.matmul(of_ps[:, 0:P], lhsT=negWum_sb[:],
                             rhs=mfT_sb[:, 0:P], start=False, stop=False)
            nc.tensor.matmul(of_ps[:, 0:P], lhsT=ntf_sb[:, 0:H],
                             rhs=ident_sb[:], start=False, stop=True)
            outF_sb = sb.tile([H, 4 * P], F32, tag="outT")
            nc.vector.tensor_scalar(out=outF_sb[:, 0:P], in0=of_ps[:, 0:P],
                                    scalar1=0.0, scalar2=None, op0=ALU.max)
            nc.sync.dma_start(outT[:, l1:l1 + P], outF_sb[:, 0:P])

    nc.compile()
    return nc


def _host_prep(x, edge_attr, edge_hidden, W_msg, b_msg, W_upd, b_upd,
               edge_index):
    src = np.asarray(edge_index[0], dtype=np.int64)
    tgt = np.asarray(edge_index[1], dtype=np.int64)
    eh = np.asarray(edge_hidden, dtype=np.float32)
    ea = np.asarray(edge_attr, dtype=np.float32)
    x = np.asarray(x, dtype=np.float32)
    W_msg = np.asarray(W_msg, dtype=np.float32)
    b_msg = np.asarray(b_msg, dtype=np.float32)
    W_upd = np.asarray(W_upd, dtype=np.float32)
    b_upd = np.asarray(b_upd, dtype=np.float32)
    assert not np.any(b_msg), "nonzero b_msg unsupported by this build"

    # ---- tgt-sort & per-(core, block) runs ----
    order = np.argsort(tgt, kind="stable")
    tgt_s = tgt[order]
    bnd = np.empty((NC, NBLK, 2), np.int64)
    for c in range(NC):
        for b in range(NBLK):
            lo_n = c * NPC + b * P
            hi_n = min(c * NPC + (b + 1) * P, (c + 1) * NPC)
            bnd[c, b] = (np.searchsorted(tgt_s, lo_n, "left"),
                         np.searchsorted(tgt_s, hi_n, "left"))
    runs = bnd[:, :, 1] - bnd[:, :, 0]
    k_blk = int(np.ceil(runs.max() / P))
    if k_blk % 2:
        k_blk += 1                      # nch even for the 2-half packing
    nch = NBLK * k_blk
    l1 = nch * P
    hch = nch // 2

    # ---- exclusion groups (reference's int logic) ----
    keys = tgt * N + src
    q = src * N + tgt
    order2 = np.argsort(keys, kind="stable")
    sk = keys[order2]
    lo2 = np.searchsorted(sk, q, "left")
    hi2 = np.searchsorted(sk, q, "right")
    eids = np.arange(E, dtype=np.int64)
    rev = np.where(eids < E2, eids + E2, eids - E2)
    simple = (hi2 - lo2 == 1) & (order2[lo2] == rev)
    affected = np.where(~simple)[0]

    Wmsg_io = np.ascontiguousarray(W_msg.T)         # [in, out]
    Wmsg2 = np.concatenate([Wmsg_io, Wmsg_io], axis=0).astype(NPBF)
    iota_t = np.tile(np.arange(P, dtype=np.float32), (P, 1))

    in_maps = []
    meta = []
    for c in range(NC):
        gl = np.zeros(l1, np.int64)      # in-edge f per padded position
        trel = np.full(l1, -1.0, np.float32)
        valid = np.zeros(l1, bool)
        for b in range(NBLK):
            lo, hi = bnd[c, b]
            n = hi - lo
            base = b * k_blk * P
            gl[base:base + n] = order[lo:hi]
            trel[base:base + n] = tgt_s[lo:hi] - (c * NPC + b * P)
            valid[base:base + n] = True

        ehp = eh[gl].astype(NPBF)                     # [l1, 64]
        eh2 = np.empty((P, hch * P), NPBF)
        eh2[0:H] = ehp[:hch * P].T
        eh2[H:P] = ehp[hch * P:].T

        tgt_rel = np.ascontiguousarray(
            trel.reshape(nch, P).T)

        # pass 2: out-edge e = rev(f); src_e = tgt_f
        el = rev[gl]
        attr_Tc = np.ascontiguousarray(ea[el].T).astype(NPBF)
        u2 = np.zeros((P, l1), np.float32)
        pos = np.arange(l1)
        tr = trel.astype(np.int64)
        u2[tr[valid], pos[valid]] = 1.0
        u2 = u2.astype(NPBF)

        xpad = np.zeros((NPC_PAD, H), np.float32)
        n_x = min(NPC_PAD, N - c * NPC)
        xpad[:n_x] = x[c * NPC:c * NPC + n_x]

        # corrections
        aff_c = affected[(src[affected] >= c * NPC)
                         & (src[affected] < (c + 1) * NPC)]
        f_list, s_cols = [], []
        for d, e in enumerate(aff_c):
            for f in order2[lo2[e]:hi2[e]]:
                if f != rev[e]:
                    f_list.append(f)
                    s_cols.append(d)
        assert len(aff_c) <= SPEC_CAP, len(aff_c)
        assert len(f_list) <= P, len(f_list)
        ehF = np.zeros((P, H), np.float32)
        if f_list:
            ehF[:len(f_list)] = eh[np.asarray(f_list)]
        ehRF = np.zeros((P, H), np.float32)
        attrF = np.zeros((P, A), np.float32)
        if len(aff_c):
            ehRF[:len(aff_c)] = eh[rev[aff_c]]
            attrF[:len(aff_c)] = ea[aff_c]
        Sneg = np.zeros((P, P), np.float32)
        for fi, d in enumerate(s_cols):
            Sneg[fi, d] = -1.0
        didx = np.zeros((P, 1), np.int32)
        didx[:len(aff_c), 0] = src[aff_c] - c * NPC

        in_maps.append({
            "eh2": eh2,
            "tgt_rel": tgt_rel,
            "attr_T": attr_Tc,
            "U2": u2,
            "xT_own": np.ascontiguousarray(xpad.T),
            "ehF_T": np.ascontiguousarray(ehF.T).astype(NPBF),
            "ehRF_T": np.ascontiguousarray(ehRF.T).astype(NPBF),
            "attrF_T": np.ascontiguousarray(attrF.T).astype(NPBF),
            "Sneg": Sneg,
            "didx": didx,
            "Wmsg2": Wmsg2,
            "Wua": np.ascontiguousarray(W_upd[:, H:H + A].T).astype(NPBF),
            "negWum": np.ascontiguousarray(-W_upd[:, H + A:].T).astype(NPBF),
            "Wstack": np.concatenate(
                [-W_upd[:, H + A:].T, W_upd[:, H:H + A].T],
                axis=0).astype(NPBF),
            "Wum": np.ascontiguousarray(W_upd[:, H + A:].T),
            "Wux": np.ascontiguousarray(W_upd[:, :H].T),
            "bupd": np.ascontiguousarray(b_upd[None, :]),
            "ones1": np.ones((1, P), np.float32),
            "ident": np.eye(P, dtype=np.float32).astype(NPBF),
            "iota4": np.tile(iota_t, (1, 4)).astype(NPBF),
        })
        meta.append({"el": el, "valid": valid, "aff_c": aff_c})
    return in_maps, meta, k_blk


def kernel(**inputs) -> np.ndarray:
    in_maps, meta, k_blk = _host_prep(**inputs)
    if k_blk not in _CACHE:
        _CACHE[k_blk] = _build(k_blk)
    nc = _CACHE[k_blk]
    res = run_bass_kernel_spmd(nc, in_maps, core_ids=list(range(NC)))
    l1 = NBLK * k_blk * P
    out = np.empty((E, H), np.float32)
    for c in range(NC):
        oT = res.results[c]["outT"]
        m = meta[c]
        out[m["el"][m["valid"]]] = oT[:, :l1].T[m["valid"]]
    for c in range(NC):
        oT = res.results[c]["outT"]
        aff_c = meta[c]["aff_c"]
        if len(aff_c):
            out[aff_c] = oT[:, l1:l1 + len(aff_c)].T
    return out



# revision 4
# speedup vs baseline: 1.3323x; 1.3323x over previous
"""Trainium2 Bass kernel for a directed MPNN layer (8 NeuronCores, SPMD).

Reference computation (per edge e = (src, tgt)):
    msg  = relu(edge_hidden @ W_msg.T + b_msg)                     (E, H)
    agg  = segment_sum(msg, tgt, N)                                (N, H)
    excl[e] = sum msg[f] over f with (tgt_f, src_f) == (src_e, tgt_e)
    out[e]  = relu(x[src_e] @ Wx.T + edge_attr[e] @ Wa.T
                   + (agg[src_e] - excl[e]) @ Wm.T + b_upd)
  with W_upd = [Wx | Wa | Wm] split along columns (64 | 16 | 64).

Decomposition (no cross-core communication):
    node_term[v] = x[v] @ Wx.T + agg[v] @ Wm.T + b_upd
    out[e] = relu(node_term[src_e] + edge_attr[e] @ Wa.T - excl[e] @ Wm.T)

  Each core owns 5000 nodes (40 blocks of 128). Edges are reverse pairs
  (e <-> e +/- E/2), so for out-edge e = rev(f), excl[e] = msg[f] (plus
  rare duplicate-pair corrections) and src_e = tgt_f. In-edges sorted by
  tgt give one stream; everything for a 128-node block is FUSED into a
  single pass (node_term never round-trips DRAM):
    per group of 4 chunks (512 edges):
      msgT = Wmsg^T ehT          (PE)   -> relu -> sx     (ACT)
      msg  = eh Wmsg (edge-major, PE)   -> relu bf16      (DVE)
      t4 one-hot (iota == tgt)          (Pool)
      aggT += msg^T t4                  (PE)
    node_term = aggT^T Wum + x Wux + b  (PE + DVE)
    out = relu(Wstack^T [relu(msgT); attrT] + nt^T u2)  (PE + DVE)
  ~hundreds of duplicate-pair corrections go through a fix-up group
  whose outputs the host splices in (node_term blocks are also written
  to DRAM for the fix-up's indirect gather).

Layouts: edge hidden + outputs are packed 2 groups per 128 partitions
(halves alternate by group) so every big DMA moves 128-partition tiles.
Output is bf16 (host upcasts); matmul accumulation is fp32 PSUM.
"""

import numpy as np
import ml_dtypes

import concourse.bacc as bacc
import concourse.bass as bass
import concourse.mybir as mybir
import concourse.tile as tile
from concourse.bass_utils import run_bass_kernel_spmd

F32 = mybir.dt.float32
F32R = mybir.dt.float32r
BF16 = mybir.dt.bfloat16
I32 = mybir.dt.int32
ALU = mybir.AluOpType
ACTF = mybir.ActivationFunctionType
NPBF = ml_dtypes.bfloat16

N = 40000
E = 800000
E2 = E // 2
H = 64
A = 16
NC = 8
P = 128

NPC = N // NC           # 5000 nodes per core
NBLK = 40               # 128-node blocks per core
NPC_PAD = NBLK * P      # 5120
SPEC_CAP = P            # special (correction) rows per core

_CACHE = {}


def _build(k_blk: int):
    assert k_blk % 4 == 0
    ngb = k_blk // 4                # groups (of 4 chunks) per block
    cpb = (ngb + 1) // 2            # 512-col slots per block in eh/out
    wblk = cpb * 512                # eh/out columns per block
    nch = NBLK * k_blk
    l1 = nch * P

    nc = bacc.Bacc("TRN2", target_bir_lowering=False, debug=False,
                   num_devices=NC)

    def inp(name, shape, dtype):
        return nc.dram_tensor(name, shape, dtype, kind="ExternalInput").ap()

    eh_g = inp("eh_g", [P, NBLK * wblk], BF16)
    tgt_rel = inp("tgt_rel", [P, nch], BF16)
    attr_T = inp("attr_T", [A, l1], BF16)      # edge_attr of rev(f), T
    U2 = inp("U2", [P, l1], BF16)              # one-hot tgt_rel columns
    xT_own = inp("xT_own", [H, NPC_PAD], F32R)
    ehF_T = inp("ehF_T", [H, P], BF16)         # correction source rows
    ehRF_T = inp("ehRF_T", [H, P], BF16)       # eh[rev(affected e)], T
    attrF_T = inp("attrF_T", [A, P], BF16)
    Sneg = inp("Sneg", [P, P], F32R)
    didx = inp("didx", [P, 1], I32)
    Wmsg2 = inp("Wmsg2", [P, H], BF16)         # W_msg.T doubled (2x64)
    Wua = inp("Wua", [A, H], BF16)
    negWum = inp("negWum", [H, H], BF16)
    Wstack = inp("Wstack", [H + A, H], BF16)   # [negWum ; Wua]
    Wum = inp("Wum", [H, H], F32R)
    Wux = inp("Wux", [H, H], F32R)
    bupd = inp("bupd", [1, H], F32R)
    ones1 = inp("ones1", [1, P], F32R)
    ident = inp("ident", [P, P], BF16)
    iota4 = inp("iota4", [P, 4 * P], BF16)

    outD = nc.dram_tensor("outD", [P, NBLK * wblk], BF16,
                          kind="ExternalOutput").ap()
    outF = nc.dram_tensor("outF", [H, P], F32, kind="ExternalOutput").ap()
    nt_own = nc.dram_tensor("nt_own", [NPC_PAD, H], BF16).ap()

    with tile.TileContext(nc) as tc:
        with (
            tc.tile_pool(name="const", bufs=1) as cst,
            tc.tile_pool(name="peh", bufs=2) as peh,
            tc.tile_pool(name="psx", bufs=ngb + 2) as psx,
            tc.tile_pool(name="pmsg", bufs=3) as pmsg,
            tc.tile_pool(name="pt4", bufs=3) as pt4,
            tc.tile_pool(name="pu2", bufs=3) as pu2,
            tc.tile_pool(name="pnt", bufs=2) as pnt,
            tc.tile_pool(name="pout", bufs=2) as pout,
            tc.tile_pool(name="ps_mT", bufs=2, space="PSUM") as ps_mT,
            tc.tile_pool(name="ps_m4", bufs=2, space="PSUM") as ps_m4,
            tc.tile_pool(name="ps_agg", bufs=1, space="PSUM") as ps_agg,
            tc.tile_pool(name="ps_nt", bufs=1, space="PSUM") as ps_nt,
            tc.tile_pool(name="ps_o", bufs=2, space="PSUM") as ps_o,
        ):
            def load_const(name, ap_in, shape, dtype, eng=None):
                t = cst.tile(shape, dtype, tag=name)
                (eng or nc.scalar).dma_start(t[:], ap_in[:])
                return t

            tgt_rel_sb = load_const("c_tgtrel", tgt_rel, [P, nch], BF16)
            Wmsg2_sb = load_const("c_wmsg2", Wmsg2, [P, H], BF16)
            Wua_sb = load_const("c_wua", Wua, [A, H], BF16)
            negWum_sb = load_const("c_nwum", negWum, [H, H], BF16)
            Wstack_sb = load_const("c_wstack", Wstack, [H + A, H], BF16)
            Wum_sb = load_const("c_wum", Wum, [H, H], F32R)
            Wux_sb = load_const("c_wux", Wux, [H, H], F32R)
            bupd_sb = load_const("c_bupd", bupd, [1, H], F32R)
            ones1_sb = load_const("c_ones1", ones1, [1, P], F32R)
            ident_sb = load_const("c_ident", ident, [P, P], BF16)
            iota4_sb = load_const("c_iota4", iota4, [P, 4 * P], BF16)
            xT_sb = load_const("c_xt", xT_own, [H, NPC_PAD], F32R)
            Sneg_sb = load_const("c_sneg", Sneg, [P, P], F32R, nc.gpsimd)
            didx_sb = load_const("c_didx", didx, [P, 1], I32, nc.gpsimd)
            ehF_sb = load_const("c_ehf", ehF_T, [H, P], BF16, nc.gpsimd)
            ehRF_sb = load_const("c_ehrf", ehRF_T, [H, P], BF16, nc.gpsimd)
            attrF_sb = load_const("c_attrf", attrF_T, [A, P], BF16,
                                  nc.gpsimd)

            # b_upd broadcast to 128 partitions via K=1 matmul
            ps_b = ps_nt.tile([P, H], F32, tag="nt")
            nc.tensor.matmul(ps_b[:], lhsT=ones1_sb[:], rhs=bupd_sb[:],
                             start=True, stop=True)
            b_bcast = cst.tile([P, H], F32, tag="c_bb")
            nc.vector.tensor_copy(b_bcast[:], ps_b[:])

            for b in range(NBLK):
                ehb = peh.tile([P, wblk], BF16, tag="eh")
                nc.sync.dma_start(ehb[:], eh_g[:, b * wblk:(b + 1) * wblk])
                agg_ps = ps_agg.tile([H, P], F32, tag="agg")
                sx_tiles = []
                for g in range(ngb):
                    half = H * (g % 2)
                    cb = (g // 2) * 512
                    ch0 = b * k_blk + 4 * g
                    c0 = ch0 * P
                    mT_ps = ps_mT.tile([H, 512], F32, tag="mT")
                    nc.tensor.matmul(mT_ps[:],
                                     lhsT=Wmsg2_sb[half:half + H, :],
                                     rhs=ehb[half:half + H, cb:cb + 512],
                                     start=True, stop=True)
                    sx = psx.tile([H + A, 512], BF16, tag="sx")
                    nc.scalar.activation(sx[0:H, :], mT_ps[:], ACTF.Relu)
                    nc.gpsimd.dma_start(sx[H:H + A, :],
                                        attr_T[:, c0:c0 + 512])
                    sx_tiles.append(sx)
                    m4_ps = ps_m4.tile([P, 4 * H], F32, tag="m4")
                    for j in range(4):
                        nc.tensor.matmul(
                            m4_ps[:, j * H:(j + 1) * H],
                            lhsT=ehb[half:half + H,
                                     cb + j * P:cb + (j + 1) * P],
                            rhs=Wmsg2_sb[half:half + H, :],
                            start=True, stop=True)
                    m4_sb = pmsg.tile([P, 4 * H], BF16, tag="m4s")
                    nc.vector.tensor_scalar(out=m4_sb[:], in0=m4_ps[:],
                                            scalar1=0.0, scalar2=None,
                                            op0=ALU.max)
                    t4 = pt4.tile([P, 4 * P], BF16, tag="t4")
                    trs = tgt_rel_sb[:, ch0:ch0 + 4]
                    tr_bc = bass.AP(trs.tensor, trs.offset,
                                    trs.ap[:1] + [[1, 4], [0, P]])
                    nc.vector.tensor_tensor(out=t4[:], in0=iota4_sb[:],
                                            in1=tr_bc, op=ALU.is_equal)
                    for j in range(4):
                        nc.tensor.matmul(agg_ps[:],
                                         lhsT=m4_sb[:, j * H:(j + 1) * H],
                                         rhs=t4[:, j * P:(j + 1) * P],
                                         start=(g == 0 and j == 0),
                                         stop=(g == ngb - 1 and j == 3))
                # node_term for this block
                aggT_sb = pnt.tile([H, P], F32R, tag="aggT")
                nc.scalar.copy(aggT_sb[:], agg_ps[:])
                nt_ps = ps_nt.tile([P, H], F32, tag="nt")
                nc.tensor.matmul(nt_ps[:], lhsT=aggT_sb[:], rhs=Wum_sb[:],
                                 start=True, stop=False)
                nc.tensor.matmul(nt_ps[:],
                                 lhsT=xT_sb[:, b * P:(b + 1) * P],
                                 rhs=Wux_sb[:],
                                 start=False, stop=True)
                ntb = pnt.tile([P, H], BF16, tag="ntb")
                nc.vector.tensor_tensor(out=ntb[:], in0=nt_ps[:],
                                        in1=b_bcast[:], op=ALU.add)
                nc.sync.dma_start(nt_own[b * P:(b + 1) * P, :], ntb[:])
                # out for this block's edges
                ost = pout.tile([P, wblk], BF16, tag="ost")
                for g in range(ngb):
                    ch0 = b * k_blk + 4 * g
                    c0 = ch0 * P
                    u2t = pu2.tile([P, 4 * P], BF16, tag="u2")
                    nc.sync.dma_start(u2t[:], U2[:, c0:c0 + 512])
                    o_ps = ps_o.tile([H, 512], F32, tag="o")
                    nc.tensor.matmul(o_ps[:], lhsT=Wstack_sb[:],
                                     rhs=sx_tiles[g][:],
                                     start=True, stop=False)
                    nc.tensor.matmul(o_ps[:], lhsT=ntb[:], rhs=u2t[:],
                                     start=False, stop=True)
                    ro = H * (g % 2)
                    co = (g // 2) * 512
                    if g % 4 == 3:
                        nc.vector.tensor_scalar(
                            out=ost[ro:ro + H, co:co + 512], in0=o_ps[:],
                            scalar1=0.0, scalar2=None, op0=ALU.max)
                    else:
                        nc.scalar.activation(ost[ro:ro + H, co:co + 512],
                                             o_ps[:], ACTF.Relu)
                nc.scalar.dma_start(outD[:, b * wblk:(b + 1) * wblk],
                                    ost[:])

            # ---- fix-up for duplicate-pair corrected edges ----
            mF_ps = ps_mT.tile([H, 512], F32, tag="mT")
            nc.tensor.matmul(mF_ps[:, 0:P], lhsT=Wmsg2_sb[0:H, :],
                             rhs=ehF_sb[:], start=True, stop=True)
            mFT_sb = pmsg.tile([H, P], F32R, tag="mFT")
            nc.vector.tensor_scalar(out=mFT_sb[:], in0=mF_ps[:, 0:P],
                                    scalar1=0.0, scalar2=None, op0=ALU.max)
            mV_ps = ps_nt.tile([P, H], F32, tag="nt")
            nc.tensor.matmul(mV_ps[:], lhsT=mFT_sb[:], rhs=Wum_sb[:],
                             start=True, stop=True)
            mV_sb = pnt.tile([P, H], F32R, tag="mV")
            nc.vector.tensor_copy(mV_sb[:], mV_ps[:])
            ntgD_sb = pnt.tile([P, H], BF16, tag="ntg")
            nc.gpsimd.indirect_dma_start(
                out=ntgD_sb[:], out_offset=None, in_=nt_own[:],
                in_offset=bass.IndirectOffsetOnAxis(ap=didx_sb[:, 0:1],
                                                    axis=0),
            )
            ntgD_f = pnt.tile([P, H], F32, tag="ntgf")
            nc.vector.tensor_copy(ntgD_f[:], ntgD_sb[:])
            spec_ps = ps_m4.tile([P, H], F32, tag="m4")
            nc.tensor.matmul(spec_ps[:], lhsT=Sneg_sb[:], rhs=mV_sb[:],
                             start=True, stop=True)
            spec_sb = pnt.tile([P, H], BF16, tag="spec")
            nc.vector.tensor_tensor(out=spec_sb[:], in0=spec_ps[:],
                                    in1=ntgD_f[:], op=ALU.add)
            mf_ps = ps_mT.tile([H, 512], F32, tag="mT")
            nc.tensor.matmul(mf_ps[:, 0:P], lhsT=Wmsg2_sb[0:H, :],
                             rhs=ehRF_sb[:], start=True, stop=True)
            mfT_sb = pmsg.tile([H, P], BF16, tag="mrevT")
            nc.scalar.activation(mfT_sb[:], mf_ps[:, 0:P], ACTF.Relu)
            of_ps = ps_o.tile([H, 512], F32, tag="o")
            nc.tensor.matmul(of_ps[:, 0:P], lhsT=Wua_sb[:],
                             rhs=attrF_sb[:], start=True, stop=False)
            nc.tensor.matmul(of_ps[:, 0:P], lhsT=negWum_sb[:],
                             rhs=mfT_sb[:], start=False, stop=False)
            nc.tensor.matmul(of_ps[:, 0:P], lhsT=spec_sb[:],
                             rhs=ident_sb[:], start=False, stop=True)
            outF_sb = pout.tile([H, P], F32, tag="outF")
            nc.vector.tensor_scalar(out=outF_sb[:], in0=of_ps[:, 0:P],
                                    scalar1=0.0, scalar2=None, op0=ALU.max)
            nc.sync.dma_start(outF[:], outF_sb[:])

    nc.compile()
    return nc


def _host_prep(x, edge_attr, edge_hidden, W_msg, b_msg, W_upd, b_upd,
               edge_index):
    src = np.asarray(edge_index[0], dtype=np.int64)
    tgt = np.asarray(edge_index[1], dtype=np.int64)
    eh = np.asarray(edge_hidden, dtype=np.float32)
    ea = np.asarray(edge_attr, dtype=np.float32)
    x = np.asarray(x, dtype=np.float32)
    W_msg = np.asarray(W_msg, dtype=np.float32)
    b_msg = np.asarray(b_msg, dtype=np.float32)
    W_upd = np.asarray(W_upd, dtype=np.float32)
    b_upd = np.asarray(b_upd, dtype=np.float32)
    assert not np.any(b_msg), "nonzero b_msg unsupported by this build"

    # ---- tgt-sort & per-(core, block) runs ----
    order = np.argsort(tgt, kind="stable")
    tgt_s = tgt[order]
    bnd = np.empty((NC, NBLK, 2), np.int64)
    for c in range(NC):
        for b in range(NBLK):
            lo_n = c * NPC + b * P
            hi_n = min(c * NPC + (b + 1) * P, (c + 1) * NPC)
            bnd[c, b] = (np.searchsorted(tgt_s, lo_n, "left"),
                         np.searchsorted(tgt_s, hi_n, "left"))
    runs = bnd[:, :, 1] - bnd[:, :, 0]
    k_blk = int(np.ceil(runs.max() / P))
    k_blk = ((k_blk + 3) // 4) * 4          # groups of 4 chunks
    ngb = k_blk // 4
    cpb = (ngb + 1) // 2
    wblk = cpb * 512
    nch = NBLK * k_blk
    l1 = nch * P

    # ---- exclusion groups (reference's int logic) ----
    keys = tgt * N + src
    q = src * N + tgt
    order2 = np.argsort(keys, kind="stable")
    sk = keys[order2]
    lo2 = np.searchsorted(sk, q, "left")
    hi2 = np.searchsorted(sk, q, "right")
    eids = np.arange(E, dtype=np.int64)
    rev = np.where(eids < E2, eids + E2, eids - E2)
    simple = (hi2 - lo2 == 1) & (order2[lo2] == rev)
    affected = np.where(~simple)[0]

    Wmsg_io = np.ascontiguousarray(W_msg.T)         # [in, out]
    Wmsg2 = np.concatenate([Wmsg_io, Wmsg_io], axis=0).astype(NPBF)
    iota_t = np.tile(np.arange(P, dtype=np.float32), (P, 1))

    in_maps = []
    meta = []
    for c in range(NC):
        gl = np.zeros(l1, np.int64)      # in-edge f per padded position
        trel = np.full(l1, -1.0, np.float32)
        valid = np.zeros(l1, bool)
        for b in range(NBLK):
            lo, hi = bnd[c, b]
            n = hi - lo
            base = b * k_blk * P
            gl[base:base + n] = order[lo:hi]
            trel[base:base + n] = tgt_s[lo:hi] - (c * NPC + b * P)
            valid[base:base + n] = True

        ehp = eh[gl].astype(NPBF)                     # [l1, 64]
        eh_gc = np.zeros((P, NBLK * wblk), NPBF)
        for b in range(NBLK):
            for g in range(ngb):
                half = H * (g % 2)
                cols = b * wblk + (g // 2) * 512
                p0 = (b * k_blk + 4 * g) * P
                eh_gc[half:half + H, cols:cols + 512] = ehp[p0:p0 + 512].T

        tgt_rel = np.ascontiguousarray(
            trel.reshape(nch, P).T).astype(NPBF)

        # out-edge e = rev(f); src_e = tgt_f
        el = rev[gl]
        attr_Tc = np.ascontiguousarray(ea[el].T).astype(NPBF)
        u2 = np.zeros((P, l1), np.float32)
        pos = np.arange(l1)
        tr = trel.astype(np.int64)
        u2[tr[valid], pos[valid]] = 1.0
        u2 = u2.astype(NPBF)

        xpad = np.zeros((NPC_PAD, H), np.float32)
        n_x = min(NPC_PAD, N - c * NPC)
        xpad[:n_x] = x[c * NPC:c * NPC + n_x]

        # corrections
        aff_c = affected[(src[affected] >= c * NPC)
                         & (src[affected] < (c + 1) * NPC)]
        f_list, s_cols = [], []
        for d, e in enumerate(aff_c):
            for f in order2[lo2[e]:hi2[e]]:
                if f != rev[e]:
                    f_list.append(f)
                    s_cols.append(d)
        assert len(aff_c) <= SPEC_CAP, len(aff_c)
        assert len(f_list) <= P, len(f_list)
        ehF = np.zeros((P, H), np.float32)
        if f_list:
            ehF[:len(f_list)] = eh[np.asarray(f_list)]
        ehRF = np.zeros((P, H), np.float32)
        attrF = np.zeros((P, A), np.float32)
        if len(aff_c):
            ehRF[:len(aff_c)] = eh[rev[aff_c]]
            attrF[:len(aff_c)] = ea[aff_c]
        Sneg = np.zeros((P, P), np.float32)
        for fi, d in enumerate(s_cols):
            Sneg[fi, d] = -1.0
        didx = np.zeros((P, 1), np.int32)
        didx[:len(aff_c), 0] = src[aff_c] - c * NPC

        in_maps.append({
            "eh_g": eh_gc,
            "tgt_rel": tgt_rel,
            "attr_T": attr_Tc,
            "U2": u2,
            "xT_own": np.ascontiguousarray(xpad.T),
            "ehF_T": np.ascontiguousarray(ehF.T).astype(NPBF),
            "ehRF_T": np.ascontiguousarray(ehRF.T).astype(NPBF),
            "attrF_T": np.ascontiguousarray(attrF.T).astype(NPBF),
            "Sneg": Sneg,
            "didx": didx,
            "Wmsg2": Wmsg2,
            "Wua": np.ascontiguousarray(W_upd[:, H:H + A].T).astype(NPBF),
            "negWum": np.ascontiguousarray(-W_upd[:, H + A:].T).astype(NPBF),
            "Wstack": np.concatenate(
                [-W_upd[:, H + A:].T, W_upd[:, H:H + A].T],
                axis=0).astype(NPBF),
            "Wum": np.ascontiguousarray(W_upd[:, H + A:].T),
            "Wux": np.ascontiguousarray(W_upd[:, :H].T),
            "bupd": np.ascontiguousarray(b_upd[None, :]),
            "ones1": np.ones((1, P), np.float32),
            "ident": np.eye(P, dtype=np.float32).astype(NPBF),
            "iota4": np.tile(iota_t, (1, 4)).astype(NPBF),
        })
        meta.append({"el": el, "valid": valid, "aff_c": aff_c})
    return in_maps, meta, k_blk


def kernel(**inputs) -> np.ndarray:
    in_maps, meta, k_blk = _host_prep(**inputs)
    if k_blk not in _CACHE:
        _CACHE[k_blk] = _build(k_blk)
    nc = _CACHE[k_blk]
    res = run_bass_kernel_spmd(nc, in_maps, core_ids=list(range(NC)))
    ngb = k_blk // 4
    cpb = (ngb + 1) // 2
    wblk = cpb * 512
    nch = NBLK * k_blk
    l1 = nch * P
    out = np.empty((E, H), np.float32)
    for c in range(NC):
        oD = res.results[c]["outD"]
        oT = np.empty((H, l1), np.float32)
        for b in range(NBLK):
            for g in range(ngb):
                half = H * (g % 2)
                cols = b * wblk + (g // 2) * 512
                p0 = (b * k_blk + 4 * g) * P
                oT[:, p0:p0 + 512] = oD[half:half + H, cols:cols + 512]
        m = meta[c]
        out[m["el"][m["valid"]]] = oT.T[m["valid"]]
    for c in range(NC):
        oF = res.results[c]["outF"]
        aff_c = meta[c]["aff_c"]
        if len(aff_c):
            out[aff_c] = np.asarray(oF[:, :len(aff_c)].T, np.float32)
    return out
